# revision 1
# baseline (speedup 1.0000x reference)
"""Multi-head attention (B=4, T=S=2048, E=1024, H=16, D=64) on 8 TRN2 NeuronCores.

Sharding: core c handles batch b=c//2 and head-group g=c%2 (8 of 16 heads).
Each core computes its 8 heads' attention plus the matching column-slice of
the output projection, producing a partial [T, E] f32 output. Host sums the
two partials per batch and adds bo.

On-chip dataflow (all matmuls bf16 with fp32 PSUM accumulation):
  qT[d,t] = WqT.T @ queryT       (d-major projections, per 128-dim head pair)
  kT[d,t] likewise; v[s,d] natural via value.T as the stationary operand
  S.T[s,t] = kT_h.T @ qT_h       (two heads row-packed in the 128-row PE array)
  expS.T   = exp(S.T * 1/8)      (ScalarE, PSUM -> SBUF bf16)
  [O.T;den]= [v_h|1].T @ expS.T  (ones-augmented stationary -> denominators)
  Onorm    = O.T * (1/den)       (reciprocal + GPSIMD partition broadcast)
  partial  = Onorm.T @ WoSlice   (accumulate over the core's 4 head pairs)

Emission is software-pipelined: stage s=(pair, t-quarter) in pair-major
order; each stage's 16 score-tile slots interleave the previous stage's PV
accumulation plus spread-out projection / v-projection / out-projection
work, keeping ScalarE (the exp bottleneck) continuously fed.
"""

from contextlib import ExitStack

import numpy as np
import ml_dtypes

B, T, S, E = 4, 2048, 2048, 1024
H, D = 16, 64
DC = 512          # dims per core (8 heads x 64)
NP = 4            # head pairs per core
NS = S // 128     # 16 s-tiles
NQ = 4            # t-quarters of 512

_BF16 = ml_dtypes.bfloat16

_cached = None


def _build(repeats=1):
    import concourse.bass as bass
    import concourse.mybir as mybir
    import concourse.tile as tile
    from concourse import bacc

    f32 = mybir.dt.float32
    bf16 = mybir.dt.bfloat16
    AF = mybir.ActivationFunctionType

    nc = bacc.Bacc("TRN2", target_bir_lowering=False)

    qT_d = nc.dram_tensor("qT", [E, T], bf16, kind="ExternalInput")
    kT_d = nc.dram_tensor("kT", [E, S], bf16, kind="ExternalInput")
    vT_d = nc.dram_tensor("vT", [E, S], bf16, kind="ExternalInput")
    WqT_d = nc.dram_tensor("WqT", [E, DC], bf16, kind="ExternalInput")
    WkT_d = nc.dram_tensor("WkT", [E, DC], bf16, kind="ExternalInput")
    WvT_d = nc.dram_tensor("WvT", [E, DC], bf16, kind="ExternalInput")
    WoS_d = nc.dram_tensor("WoS", [DC, E], bf16, kind="ExternalInput")
    bq_d = nc.dram_tensor("bq", [128, NP], f32, kind="ExternalInput")
    bk_d = nc.dram_tensor("bk", [128, NP], f32, kind="ExternalInput")
    bv_d = nc.dram_tensor("bv", [1, DC], f32, kind="ExternalInput")
    out_d = nc.dram_tensor("out", [T, E], f32, kind="ExternalOutput")

    with tile.TileContext(nc) as tc, ExitStack() as ctx:
        persist = ctx.enter_context(tc.tile_pool(name="persist", bufs=1))
        psc = ctx.enter_context(tc.tile_pool(name="psc", bufs=2, space="PSUM"))
        ppv = ctx.enter_context(tc.tile_pool(name="ppv", bufs=2, space="PSUM"))
        pmx = ctx.enter_context(tc.tile_pool(name="pmx", bufs=2, space="PSUM"))
        expool = ctx.enter_context(tc.tile_pool(name="expool", bufs=22))
        small = ctx.enter_context(tc.tile_pool(name="small", bufs=3))
        ocp_pool = ctx.enter_context(tc.tile_pool(name="ocp", bufs=3))
        xin = ctx.enter_context(tc.tile_pool(name="xin", bufs=14))
        wpool = ctx.enter_context(tc.tile_pool(name="wts", bufs=24))

        # ---- persistent SBUF tiles ----
        qTs = [persist.tile([128, T], bf16, tag=f"qT{p}", name=f"qT{p}") for p in range(NP)]
        kTs = [persist.tile([128, S], bf16, tag=f"kT{p}", name=f"kT{p}") for p in range(NP)]
        vaug = [persist.tile([128, 8 * 65], bf16, tag=f"va{st}", name=f"va{st}") for st in range(NS)]
        WoSs = [persist.tile([128, E], bf16, tag=f"wo{p}", name=f"wo{p}") for p in range(NP)]
        Onorm = [persist.tile([128, T], bf16, tag=f"on{p}", name=f"on{p}") for p in range(NP)]
        bq_sb = persist.tile([128, NP], f32, tag="bq", name="bq_sb")
        bk_sb = persist.tile([128, NP], f32, tag="bk", name="bk_sb")
        bv_sb = persist.tile([128, DC], f32, tag="bv", name="bv_sb")

        nc.sync.dma_start(out=bq_sb, in_=bq_d[:, :])
        nc.sync.dma_start(out=bk_sb, in_=bk_d[:, :])
        bv_ap = bv_d[:, :]
        bv_bcast_ap = bass.AP(
            tensor=bv_ap.tensor,
            offset=bv_ap.offset,
            ap=[[0, 128], bv_ap.ap[-1]],
        )
        nc.sync.dma_start(out=bv_sb, in_=bv_bcast_ap)
        for p in range(NP):
            nc.sync.dma_start(out=WoSs[p], in_=WoS_d[p * 128:(p + 1) * 128, :])
        for st in range(NS):
            va3 = vaug[st].rearrange("p (h x) -> p h x", x=65)
            nc.vector.memset(va3[:, :, 64:65], 1.0)

        def load_wtiles(dram):
            ts_ = []
            for e in range(8):
                t_ = wpool.tile([128, DC], bf16, tag="w", name="wt")
                nc.sync.dma_start(out=t_, in_=dram[e * 128:(e + 1) * 128, :])
                ts_.append(t_)
            return ts_

        def proj_thunks(p, x_dram, w_tiles, dst, bias_sb, halves=(0, 1),
                        preload=False):
            """One pair's q/k projection as a thunk list: two column-halves;
            per half, stream 8 e-tile chunks (load + 2 quarter-MMs each),
            then bias-add the two finished quarters out of PSUM. With
            preload=True all 8 DMAs are issued before the first matmul
            (used for the serial startup blocks)."""
            thunks = []
            for half in halves:
                ps_pair = []  # the two quarter psums of this half (alloc lazily)
                xh = []

                def open_half(half=half, ps_pair=ps_pair, xh=xh):
                    for qi in range(2):
                        ps_pair.append(pmx.tile([128, 512], f32, tag="mx", name="mx_ps"))
                    if preload:
                        for e in range(8):
                            xt = xin.tile([128, 1024], bf16, tag="xin", name="xin")
                            nc.sync.dma_start(
                                out=xt,
                                in_=x_dram[e * 128:(e + 1) * 128,
                                           half * 1024:(half + 1) * 1024])
                            xh.append(xt)

                thunks.append(open_half)

                for e in range(8):
                    def echunk(e=e, half=half, ps_pair=ps_pair, xh=xh):
                        if preload:
                            xt = xh[e]
                        else:
                            xt = xin.tile([128, 1024], bf16, tag="xin", name="xin")
                            nc.sync.dma_start(
                                out=xt,
                                in_=x_dram[e * 128:(e + 1) * 128,
                                           half * 1024:(half + 1) * 1024])
                        for qi in range(2):
                            nc.tensor.matmul(
                                ps_pair[qi],
                                w_tiles[e][:, p * 128:(p + 1) * 128],
                                xt[:, qi * 512:(qi + 1) * 512],
                                start=(e == 0),
                                stop=(e == 7),
                            )
                    thunks.append(echunk)

                def close_half(half=half, ps_pair=ps_pair):
                    for qi in range(2):
                        q = half * 2 + qi
                        nc.vector.tensor_scalar_add(
                            dst[:, q * 512:(q + 1) * 512],
                            ps_pair[qi], bias_sb[:, p:p + 1])
                    ps_pair.clear()

                thunks.append(close_half)
            return thunks

        def vproj_thunks(wv_tiles, dh):
            """V projection for head-quad dh (4 heads, N=256), streamed in
            two s-halves. dh=0 feeds pairs 0-1 (needed by stage 1); dh=1
            feeds pairs 2-3 (needed from stage 9) and can spread late."""
            thunks = []
            for half in range(2):
                vh = []

                def load_half(half=half, vh=vh):
                    for e in range(8):
                        vt = xin.tile([128, 1024], bf16, tag="xin", name="xin")
                        nc.sync.dma_start(
                            out=vt,
                            in_=vT_d[e * 128:(e + 1) * 128,
                                     half * 1024:(half + 1) * 1024])
                        vh.append(vt)

                thunks.append(load_half)
                for sti in range(8):
                    def vst(sti=sti, half=half, vh=vh):
                        st = half * 8 + sti
                        # stages with live PV accumulators own the "pv"
                        # slots -> everything except (dh0, s-half0) uses "mx"
                        if dh == 0 and half == 0:
                            ps = ppv.tile([128, 512], f32, tag="pv", name="pv_ps")
                        else:
                            ps = pmx.tile([128, 512], f32, tag="mx", name="mx_ps")
                        for e in range(8):
                            nc.tensor.matmul(
                                ps[:, 0:256],
                                vh[e][:, sti * 128:(sti + 1) * 128],
                                wv_tiles[e][:, dh * 256:(dh + 1) * 256],
                                start=(e == 0),
                                stop=(e == 7),
                            )
                        va3 = vaug[st].rearrange("p (h x) -> p h x", x=65)
                        nc.vector.tensor_add(
                            va3[:, dh * 4:(dh + 1) * 4, 0:64],
                            ps[:, 0:256].rearrange("p (h x) -> p h x", x=64),
                            bv_sb[:, dh * 256:(dh + 1) * 256].rearrange(
                                "p (h x) -> p h x", x=64),
                        )
                        if half == 0 and sti == 7:
                            vh.clear()
                    thunks.append(vst)
            return thunks

        def outproj_thunks(tq):
            thunks = []
            for tt in range(tq * 4, tq * 4 + 4):
                for c in range(2):
                    def unit(tt=tt, c=c):
                        op_ps = pmx.tile([128, 512], f32, tag="mx", name="mx_ps")
                        for p in range(NP):
                            nc.tensor.matmul(
                                op_ps,
                                Onorm[p][:, tt * 128:(tt + 1) * 128],
                                WoSs[p][:, c * 512:(c + 1) * 512],
                                start=(p == 0),
                                stop=(p == 3),
                            )
                        oc = ocp_pool.tile([128, 512], f32, tag="ocp", name="oc")
                        nc.vector.tensor_copy(oc, op_ps)
                        nc.sync.dma_start(
                            out=out_d[tt * 128:(tt + 1) * 128,
                                      c * 512:(c + 1) * 512],
                            in_=oc)
                    thunks.append(unit)
            return thunks

        def outproj_tail(tq):
            # tail variant: "sc" psum tiles are free once scoring has ended,
            # so use wide [128,1024] units to avoid mx-slot serialization
            for tt in range(tq * 4, tq * 4 + 4):
                op_ps = psc.tile([128, 1024], f32, tag="sc", name="sc_ps")
                for c in range(2):
                    for p in range(NP):
                        nc.tensor.matmul(
                            op_ps[:, c * 512:(c + 1) * 512],
                            Onorm[p][:, tt * 128:(tt + 1) * 128],
                            WoSs[p][:, c * 512:(c + 1) * 512],
                            start=(p == 0),
                            stop=(p == 3),
                        )
                oc = ocp_pool.tile([128, 1024], f32, tag="ocpw", name="ocw")
                nc.vector.tensor_copy(oc, op_ps)
                nc.sync.dma_start(out=out_d[tt * 128:(tt + 1) * 128, :], in_=oc)

        class PrevStage:
            def __init__(self, p, tq, exs):
                self.p, self.tq, self.exs = p, tq, exs
                self.o_ps = None

        def emit_pv_mm(prev, h, st):
            if prev.o_ps is None:
                prev.o_ps = [None, None]
            if prev.o_ps[h] is None:
                prev.o_ps[h] = ppv.tile([128, 512], f32, tag="pv", name="pv_ps")
            hidx = 2 * prev.p + h
            nc.tensor.matmul(
                prev.o_ps[h][0:65, :],
                vaug[st][:, hidx * 65:hidx * 65 + 65],
                prev.exs[st][:, h * 512:(h + 1) * 512],
                start=(st == 0),
                stop=(st == 15),
            )

        def emit_pv_slot(prev, st):
            emit_pv_mm(prev, 0, st)
            emit_pv_mm(prev, 1, st)

        def emit_normalize(prev):
            t0 = prev.tq * 512
            for h in range(2):
                o_ps = prev.o_ps[h]
                # stage O out of PSUM immediately so the pv slots free after
                # two short DVE ops instead of the whole bcast chain; the
                # final multiply then runs in DVE 4x bf16 mode (all SBUF).
                rc = small.tile([1, 512], f32, tag="rc", name="rc")
                nc.vector.reciprocal(rc, o_ps[64:65, :])
                ocp = small.tile([64, 512], f32, tag="oc2", name="oc2")
                nc.vector.tensor_copy(ocp, o_ps[0:64, :])
                rcb = small.tile([1, 512], bf16, tag="rcb", name="rcb")
                nc.vector.tensor_copy(rcb, rc)
                rb_sb = small.tile([64, 512], bf16, tag="rb", name="rb")
                nc.gpsimd.partition_broadcast(rb_sb, rcb[0:1, :])
                nc.vector.tensor_mul(
                    Onorm[prev.p][h * 64:(h + 1) * 64, t0:t0 + 512],
                    ocp,
                    rb_sb,
                )

        def emit_stage(p, tq, prev, extras, dl=6):
            """16 score slots for (p, tq); interleave prev stage's PV and
            the extra thunks (all emitted by slot `dl`); returns this
            stage's PrevStage record."""
            t0 = tq * 512
            exs = []
            n_ex = len(extras)
            taken = 0
            for st in range(NS):
                sc_ps = psc.tile([128, 1024], f32, tag="sc", name="sc_ps")
                nc.tensor.matmul(
                    sc_ps[:, 0:512],
                    kTs[p][0:64, st * 128:(st + 1) * 128],
                    qTs[p][0:64, t0:t0 + 512],
                    start=True, stop=True,
                    tile_position=(0, 0),
                )
                nc.tensor.matmul(
                    sc_ps[:, 512:1024],
                    kTs[p][64:128, st * 128:(st + 1) * 128],
                    qTs[p][64:128, t0:t0 + 512],
                    start=True, stop=True,
                    tile_position=(64, 0),
                )
                ex = expool.tile([128, 1024], bf16, tag="ex", name="ex")
                nc.scalar.activation(ex, sc_ps, AF.Exp, scale=0.125)
                exs.append(ex)
                if prev is not None:
                    emit_pv_slot(prev, st)
                want = (n_ex * min(st + 1, dl) + dl - 1) // dl
                while taken < want:
                    extras[taken]()
                    taken += 1
            while taken < n_ex:
                extras[taken]()
                taken += 1
            if prev is not None:
                emit_normalize(prev)
            return PrevStage(p, tq, exs)

        # ---- emission ----
        for _rep in range(repeats):
            # startup: only the first column-halves of pair-0's q/k
            # projections block the first scores; everything else overlaps.
            wq_tiles = load_wtiles(WqT_d)
            q0h0 = proj_thunks(0, qT_d, wq_tiles, qTs[0], bq_sb,
                               halves=(0,), preload=True)
            wk_tiles = load_wtiles(WkT_d)
            k0h0 = proj_thunks(0, kT_d, wk_tiles, kTs[0], bk_sb,
                               halves=(0,), preload=True)
            # issue both halves' preload DMAs before any matmul runs
            q0h0[0](); k0h0[0]()
            for th in q0h0[1:]:
                th()
            for th in k0h0[1:]:
                th()
            q0 = proj_thunks(0, qT_d, wq_tiles, qTs[0], bq_sb, halves=(1,))
            k0 = proj_thunks(0, kT_d, wk_tiles, kTs[0], bk_sb, halves=(1,))
            wv_tiles = load_wtiles(WvT_d)

            # per-stage extra work, placed just-in-time:
            #  stage 0: K0/Q0 second halves + V-projection first s-half
            #  stage 1: V-projection second s-half
            #  pair p>=1: K-half0 @4p-2, Q-half0 @4p-1, K-half1 @4p, Q-half1 @4p+1
            #  stages 14, 15: out-proj for t0, t1
            vpA = vproj_thunks(wv_tiles, 0)
            extras = {0: k0 + q0 + vpA[:10], 1: vpA[10:]}
            for p in range(1, NP):
                qp = proj_thunks(p, qT_d, wq_tiles, qTs[p], bq_sb)
                kp = proj_thunks(p, kT_d, wk_tiles, kTs[p], bk_sb)
                for sg, th in ((4 * p - 2, kp[:10]), (4 * p - 1, qp[:10]),
                               (4 * p, kp[10:]), (4 * p + 1, qp[10:])):
                    extras[sg] = extras.get(sg, []) + th
            # second head-quad of V, appended after each stage's proj work
            # (sequential mx-slot handoff, done well before stage 9 needs
            # it). Skip stages 4/5 whose proj work has a hard slot-8
            # deadline (dl=7) — vpB has no deadline and shouldn't compete.
            vpB = vproj_thunks(wv_tiles, 1)
            n4 = (len(vpB) + 3) // 4
            for i, sg in enumerate((2, 3, 6, 7)):
                extras[sg] = extras.get(sg, []) + vpB[i * n4:(i + 1) * n4]
            extras[14] = extras.get(14, []) + outproj_thunks(0)
            extras[15] = extras.get(15, []) + outproj_thunks(1)

            # pacing deadlines: K-half1 stages (4p) must finish extras by
            # slot 8 (their own scores need those kT columns); stages 0/1
            # feed vaug just-in-time; elsewhere spread smoothly.
            dls = {0: 14, 1: 14, 4: 7, 8: 7, 12: 7}
            prev = None
            for s in range(16):
                p, tq = s // 4, s % 4
                prev = emit_stage(p, tq, prev, extras.get(s, []),
                                  dl=dls.get(s, 16))

            # tail: PV of the last stage with out-proj(t2) interleaved
            # (its Onorm slices completed at the end of stage 15), then the
            # final normalize and out-proj(t3)
            op2 = outproj_thunks(2)
            for st in range(NS):
                emit_pv_slot(prev, st)
                if st % 2 == 1:
                    op2[st // 2]()
            emit_normalize(prev)
            outproj_tail(3)

    nc.compile()
    return nc


def _get_nc():
    global _cached
    if _cached is None:
        _cached = _build()
    return _cached


def _prep_core_inputs(c, query, key, value, Wq, Wk, Wv, Wo, bq, bk, bv,
                      _cache={}):
    b, g = c // 2, c % 2
    sl = slice(g * DC, (g + 1) * DC)
    key_ = (id(query), b)
    if key_ not in _cache:
        # both cores of a batch share the transposed/cast activations
        _cache.clear()
        _cache[key_] = {
            "qT": query[b].T.astype(_BF16),
            "kT": key[b].T.astype(_BF16),
            "vT": value[b].T.astype(_BF16),
        }
    shared = _cache[key_]
    return {
        **shared,
        "WqT": Wq[sl].T.astype(_BF16),
        "WkT": Wk[sl].T.astype(_BF16),
        "WvT": Wv[sl].T.astype(_BF16),
        "WoS": Wo[:, sl].T.astype(_BF16),
        "bq": np.ascontiguousarray(bq[sl].reshape(NP, 128).T),
        "bk": np.ascontiguousarray(bk[sl].reshape(NP, 128).T),
        "bv": np.ascontiguousarray(bv[sl].reshape(1, DC)),
    }


def kernel(**inputs):
    from concourse.bass_utils import run_bass_kernel_spmd

    args = {k: np.asarray(inputs[k], np.float32)
            for k in ("query", "key", "value", "Wq", "Wk", "Wv", "Wo",
                      "bq", "bk", "bv", "bo")}
    _prep_core_inputs.__defaults__[0].clear()
    nc = _get_nc()
    in_maps = [
        _prep_core_inputs(c, args["query"], args["key"], args["value"],
                          args["Wq"], args["Wk"], args["Wv"], args["Wo"],
                          args["bq"], args["bk"], args["bv"])
        for c in range(8)
    ]
    res = run_bass_kernel_spmd(nc, in_maps, core_ids=list(range(8)))
    outs = [r["out"] for r in res.results]
    final = np.empty((B, T, E), np.float32)
    for b in range(B):
        final[b] = outs[2 * b] + outs[2 * b + 1] + args["bo"][None, :]
    return final



# revision 3
# speedup vs baseline: 1.0743x; 1.0743x over previous
"""Multi-head attention (B=4, T=S=2048, E=1024, H=16, D=64) on 8 TRN2 NeuronCores.

Sharding: core c handles batch b=c//2 and head-group g=c%2 (8 of 16 heads).
Each core computes its 8 heads' attention plus the matching column-slice of
the output projection, producing a partial [T, E] f32 output. Host sums the
two partials per batch and adds bo.

On-chip dataflow (all matmuls bf16 with fp32 PSUM accumulation):
  qT[d,t] = WqT.T @ queryT       (d-major projections, per 128-dim head pair)
  kT[d,t] likewise; v[s,d] natural via value.T as the stationary operand
  S.T[s,t] = kT_h.T @ qT_h       (two heads row-packed in the 128-row PE array)
  expS.T   = exp(S.T * 1/8)      (ScalarE, PSUM -> SBUF bf16)
  [O;den]  = expS_blk.T @ [v_h|1]  (exp block stationary, narrow 65-col v
                                    moving -> O in [t,d] layout + per-t dens)
  Onorm_td = O * (1/den)         (den is per-PARTITION here: one cheap DVE
                                  tensor_tensor, no cross-partition bcast)
  OnormT   = XBAR DMA-transpose of Onorm_td -> d-major   (no PE/PSUM cost)
  partial  = OnormT.T @ WoSlice  (accumulate over the core's 4 head pairs)

The [t,d]-output PV orientation keeps the PE's moving operand narrow (65
cols vs 512), halving PV time; denominators ride along as a 65th moving
column of ones, and the d-major layout the out-projection needs is restored
by the DMA crossbar instead of the PE.

Emission is software-pipelined: stage s=(pair, t-quarter) in pair-major
order; each stage's 16 score-tile slots interleave the previous stage's PV
accumulation plus spread-out projection / v-projection / out-projection
work, keeping ScalarE (the exp bottleneck) continuously fed.
"""

from contextlib import ExitStack

import numpy as np
import ml_dtypes

B, T, S, E = 4, 2048, 2048, 1024
H, D = 16, 64
DC = 512          # dims per core (8 heads x 64)
NP = 4            # head pairs per core
NS = S // 128     # 16 s-tiles
NQ = 4            # t-quarters of 512

_BF16 = ml_dtypes.bfloat16

_cached = None


def _build(repeats=1):
    import concourse.bass as bass
    import concourse.mybir as mybir
    import concourse.tile as tile
    from concourse import bacc

    f32 = mybir.dt.float32
    bf16 = mybir.dt.bfloat16
    AF = mybir.ActivationFunctionType

    nc = bacc.Bacc("TRN2", target_bir_lowering=False)

    qT_d = nc.dram_tensor("qT", [E, T], bf16, kind="ExternalInput")
    kT_d = nc.dram_tensor("kT", [E, S], bf16, kind="ExternalInput")
    vT_d = nc.dram_tensor("vT", [E, S], bf16, kind="ExternalInput")
    WqT_d = nc.dram_tensor("WqT", [E, DC], bf16, kind="ExternalInput")
    WkT_d = nc.dram_tensor("WkT", [E, DC], bf16, kind="ExternalInput")
    WvT_d = nc.dram_tensor("WvT", [E, DC], bf16, kind="ExternalInput")
    WoS_d = nc.dram_tensor("WoS", [DC, E], bf16, kind="ExternalInput")
    bq_d = nc.dram_tensor("bq", [128, NP], f32, kind="ExternalInput")
    bk_d = nc.dram_tensor("bk", [128, NP], f32, kind="ExternalInput")
    bv_d = nc.dram_tensor("bv", [1, DC], f32, kind="ExternalInput")
    out_d = nc.dram_tensor("out", [T, E], f32, kind="ExternalOutput")

    with tile.TileContext(nc) as tc, ExitStack() as ctx:
        persist = ctx.enter_context(tc.tile_pool(name="persist", bufs=1))
        psc = ctx.enter_context(tc.tile_pool(name="psc", bufs=2, space="PSUM"))
        ppv = ctx.enter_context(tc.tile_pool(name="ppv", bufs=1, space="PSUM"))
        pmx = ctx.enter_context(tc.tile_pool(name="pmx", bufs=2, space="PSUM"))
        expool = ctx.enter_context(tc.tile_pool(name="expool", bufs=22))
        small = ctx.enter_context(tc.tile_pool(name="small", bufs=2))
        ocp_pool = ctx.enter_context(tc.tile_pool(name="ocp", bufs=3))
        xin = ctx.enter_context(tc.tile_pool(name="xin", bufs=14))
        wpool = ctx.enter_context(tc.tile_pool(name="wts", bufs=24))

        # ---- persistent SBUF tiles ----
        qTs = [persist.tile([128, T], bf16, tag=f"qT{p}", name=f"qT{p}") for p in range(NP)]
        kTs = [persist.tile([128, S], bf16, tag=f"kT{p}", name=f"kT{p}") for p in range(NP)]
        vaug = [persist.tile([128, 8 * 65], bf16, tag=f"va{st}", name=f"va{st}") for st in range(NS)]
        WoSs = [persist.tile([128, E], bf16, tag=f"wo{p}", name=f"wo{p}") for p in range(NP)]
        Onorm = [persist.tile([128, T], bf16, tag=f"on{p}", name=f"on{p}") for p in range(NP)]
        bq_sb = persist.tile([128, NP], f32, tag="bq", name="bq_sb")
        bk_sb = persist.tile([128, NP], f32, tag="bk", name="bk_sb")
        bv_sb = persist.tile([128, DC], f32, tag="bv", name="bv_sb")

        nc.sync.dma_start(out=bq_sb, in_=bq_d[:, :])
        nc.sync.dma_start(out=bk_sb, in_=bk_d[:, :])
        bv_ap = bv_d[:, :]
        bv_bcast_ap = bass.AP(
            tensor=bv_ap.tensor,
            offset=bv_ap.offset,
            ap=[[0, 128], bv_ap.ap[-1]],
        )
        nc.sync.dma_start(out=bv_sb, in_=bv_bcast_ap)
        for p in range(NP):
            nc.sync.dma_start(out=WoSs[p], in_=WoS_d[p * 128:(p + 1) * 128, :])
        for st in range(NS):
            va3 = vaug[st].rearrange("p (h x) -> p h x", x=65)
            nc.vector.memset(va3[:, :, 64:65], 1.0)

        def load_wtiles(dram):
            ts_ = []
            for e in range(8):
                t_ = wpool.tile([128, DC], bf16, tag="w", name="wt")
                nc.sync.dma_start(out=t_, in_=dram[e * 128:(e + 1) * 128, :])
                ts_.append(t_)
            return ts_

        def proj_thunks(p, x_dram, w_tiles, dst, bias_sb, halves=(0, 1),
                        preload=False):
            """One pair's q/k projection as a thunk list: two column-halves;
            per half, stream 8 e-tile chunks (load + 2 quarter-MMs each),
            then bias-add the two finished quarters out of PSUM. With
            preload=True all 8 DMAs are issued before the first matmul
            (used for the serial startup blocks)."""
            thunks = []
            for half in halves:
                ps_pair = []  # the two quarter psums of this half (alloc lazily)
                xh = []

                def open_half(half=half, ps_pair=ps_pair, xh=xh):
                    for qi in range(2):
                        ps_pair.append(pmx.tile([128, 512], f32, tag="mx", name="mx_ps"))
                    if preload:
                        for e in range(8):
                            xt = xin.tile([128, 1024], bf16, tag="xin", name="xin")
                            nc.sync.dma_start(
                                out=xt,
                                in_=x_dram[e * 128:(e + 1) * 128,
                                           half * 1024:(half + 1) * 1024])
                            xh.append(xt)

                thunks.append(open_half)

                for e in range(8):
                    def echunk(e=e, half=half, ps_pair=ps_pair, xh=xh):
                        if preload:
                            xt = xh[e]
                        else:
                            xt = xin.tile([128, 1024], bf16, tag="xin", name="xin")
                            nc.sync.dma_start(
                                out=xt,
                                in_=x_dram[e * 128:(e + 1) * 128,
                                           half * 1024:(half + 1) * 1024])
                        for qi in range(2):
                            nc.tensor.matmul(
                                ps_pair[qi],
                                w_tiles[e][:, p * 128:(p + 1) * 128],
                                xt[:, qi * 512:(qi + 1) * 512],
                                start=(e == 0),
                                stop=(e == 7),
                            )
                    thunks.append(echunk)

                def close_half(half=half, ps_pair=ps_pair):
                    for qi in range(2):
                        q = half * 2 + qi
                        nc.vector.tensor_scalar_add(
                            dst[:, q * 512:(q + 1) * 512],
                            ps_pair[qi], bias_sb[:, p:p + 1])
                    ps_pair.clear()

                thunks.append(close_half)
            return thunks

        def vproj_thunks(wv_tiles, dh):
            """V projection for head-quad dh (4 heads, N=256), streamed in
            two s-halves. dh=0 feeds pairs 0-1 (needed by stage 1); dh=1
            feeds pairs 2-3 (needed from stage 9) and can spread late."""
            thunks = []
            for half in range(2):
                vh = []

                def load_half(half=half, vh=vh):
                    for e in range(8):
                        vt = xin.tile([128, 1024], bf16, tag="xin", name="xin")
                        nc.sync.dma_start(
                            out=vt,
                            in_=vT_d[e * 128:(e + 1) * 128,
                                     half * 1024:(half + 1) * 1024])
                        vh.append(vt)

                thunks.append(load_half)
                for sti in range(8):
                    def vst(sti=sti, half=half, vh=vh):
                        st = half * 8 + sti
                        ps = pmx.tile([128, 512], f32, tag="mx", name="mx_ps")
                        for e in range(8):
                            nc.tensor.matmul(
                                ps[:, 0:256],
                                vh[e][:, sti * 128:(sti + 1) * 128],
                                wv_tiles[e][:, dh * 256:(dh + 1) * 256],
                                start=(e == 0),
                                stop=(e == 7),
                            )
                        va3 = vaug[st].rearrange("p (h x) -> p h x", x=65)
                        nc.vector.tensor_add(
                            va3[:, dh * 4:(dh + 1) * 4, 0:64],
                            ps[:, 0:256].rearrange("p (h x) -> p h x", x=64),
                            bv_sb[:, dh * 256:(dh + 1) * 256].rearrange(
                                "p (h x) -> p h x", x=64),
                        )
                        if half == 0 and sti == 7:
                            vh.clear()
                    thunks.append(vst)
            return thunks

        def outproj_thunks(tq):
            thunks = []
            for tt in range(tq * 4, tq * 4 + 4):
                for c in range(2):
                    def unit(tt=tt, c=c):
                        op_ps = pmx.tile([128, 512], f32, tag="mx", name="mx_ps")
                        for p in range(NP):
                            nc.tensor.matmul(
                                op_ps,
                                Onorm[p][:, tt * 128:(tt + 1) * 128],
                                WoSs[p][:, c * 512:(c + 1) * 512],
                                start=(p == 0),
                                stop=(p == 3),
                            )
                        oc = ocp_pool.tile([128, 512], f32, tag="ocp", name="oc")
                        nc.vector.tensor_copy(oc, op_ps)
                        nc.sync.dma_start(
                            out=out_d[tt * 128:(tt + 1) * 128,
                                      c * 512:(c + 1) * 512],
                            in_=oc)
                    thunks.append(unit)
            return thunks

        def outproj_tail(tq):
            # tail variant: "sc" psum tiles are free once scoring has ended,
            # so use wide [128,1024] units to avoid mx-slot serialization
            for tt in range(tq * 4, tq * 4 + 4):
                op_ps = psc.tile([128, 1024], f32, tag="sc", name="sc_ps")
                for c in range(2):
                    for p in range(NP):
                        nc.tensor.matmul(
                            op_ps[:, c * 512:(c + 1) * 512],
                            Onorm[p][:, tt * 128:(tt + 1) * 128],
                            WoSs[p][:, c * 512:(c + 1) * 512],
                            start=(p == 0),
                            stop=(p == 3),
                        )
                oc = ocp_pool.tile([128, 1024], f32, tag="ocpw", name="ocw")
                nc.vector.tensor_copy(oc, op_ps)
                nc.sync.dma_start(out=out_d[tt * 128:(tt + 1) * 128, :], in_=oc)

        class PrevStage:
            def __init__(self, p, tq, exs):
                self.p, self.tq, self.exs = p, tq, exs
                self.pv = None      # two [128, 4*65] psum tiles (4 units each)

        def emit_pv_slot(prev, st):
            """8 accumulating matmuls: O_td[t, d]/den for each (tb, h) unit,
            with exp (s x t block) stationary and the 65-col augmented v
            moving. Unit g = tb*2 + h lives at cols (g%4)*65 of pv tile g//4;
            one start per PSUM bank (unit 0 of each tile, which pending-zeroes
            the whole bank), one stop (unit 3)."""
            if st == 0:
                prev.pv = [
                    ppv.tile([128, 260], f32, tag="pvA", name="pvA",
                             padded_shape=[128, 512]),
                    ppv.tile([128, 260], f32, tag="pvB", name="pvB",
                             padded_shape=[128, 512]),
                ]
            for g in range(8):
                tb, h = g // 2, g % 2
                pvt = prev.pv[g // 4]
                u = g % 4
                hidx = 2 * prev.p + h
                nc.tensor.matmul(
                    pvt[:, u * 65:(u + 1) * 65],
                    prev.exs[st][:, h * 512 + tb * 128:h * 512 + (tb + 1) * 128],
                    vaug[st][:, hidx * 65:hidx * 65 + 65],
                    start=(st == 0 and u == 0),
                    stop=(st == 15 and u == 3),
                )

        def emit_normalize(prev):
            """den sits on the same partition as its O row: reciprocal of the
            65th column of each unit, one tensor_tensor mul per pv tile (rc
            broadcast along d via a stride-0 AP) into a [t, d] bf16 tile, then
            one XBAR DMA transpose restoring the d-major layout out-proj
            wants (out 3D => per-128-block transpose)."""
            rc = small.tile([128, 8], f32, tag="rc", name="rc")
            otd = small.tile([128, 512], bf16, tag="otd", name="otd")
            for i in range(2):
                pv3 = prev.pv[i].rearrange("p (u x) -> p u x", x=65)
                nc.vector.reciprocal(rc[:, i * 4:(i + 1) * 4], pv3[:, :, 64:65])
            for i in range(2):
                pv3 = prev.pv[i].rearrange("p (u x) -> p u x", x=65)
                rc_b = bass.AP(
                    tensor=rc.tensor,
                    offset=rc[:, i * 4:(i + 1) * 4].offset,
                    ap=[rc.ap[0], [1, 4], [0, 64]],
                )
                nc.vector.tensor_mul(
                    otd[:, i * 256:(i + 1) * 256].rearrange(
                        "p (u x) -> p u x", x=64),
                    pv3[:, :, 0:64],
                    rc_b,
                )
            t0 = prev.tq * 512
            nc.sync.dma_start_transpose(
                out=Onorm[prev.p][:, t0:t0 + 512].rearrange(
                    "p (b t) -> p b t", t=128),
                in_=otd,
            )
            prev.pv = None

        def emit_stage(p, tq, prev, extras, dl=16, sl0=0):
            """16 score slots for (p, tq); interleave prev stage's PV and
            the extra thunks (emitted between slots sl0..sl0+dl); returns
            this stage's record."""
            t0 = tq * 512
            exs = []
            n_ex = len(extras)
            taken = 0
            for st in range(NS):
                sc_ps = psc.tile([128, 1024], f32, tag="sc", name="sc_ps")
                nc.tensor.matmul(
                    sc_ps[:, 0:512],
                    kTs[p][0:64, st * 128:(st + 1) * 128],
                    qTs[p][0:64, t0:t0 + 512],
                    start=True, stop=True,
                    tile_position=(0, 0),
                )
                nc.tensor.matmul(
                    sc_ps[:, 512:1024],
                    kTs[p][64:128, st * 128:(st + 1) * 128],
                    qTs[p][64:128, t0:t0 + 512],
                    start=True, stop=True,
                    tile_position=(64, 0),
                )
                ex = expool.tile([128, 1024], bf16, tag="ex", name="ex")
                nc.scalar.activation(ex, sc_ps, AF.Exp, scale=0.125)
                exs.append(ex)
                if st >= sl0:
                    prog = min(st - sl0 + 1, dl)
                    want = (n_ex * prog + dl - 1) // dl
                    while taken < want:
                        extras[taken]()
                        taken += 1
                if prev is not None:
                    emit_pv_slot(prev, st)
            while taken < n_ex:
                extras[taken]()
                taken += 1
            if prev is not None:
                emit_normalize(prev)
            return PrevStage(p, tq, exs)

        # ---- emission ----
        for _rep in range(repeats):
            # startup: only the first column-halves of pair-0's q/k
            # projections block the first scores; everything else overlaps.
            wq_tiles = load_wtiles(WqT_d)
            q0h0 = proj_thunks(0, qT_d, wq_tiles, qTs[0], bq_sb,
                               halves=(0,), preload=True)
            wk_tiles = load_wtiles(WkT_d)
            k0h0 = proj_thunks(0, kT_d, wk_tiles, kTs[0], bk_sb,
                               halves=(0,), preload=True)
            # issue both halves' preload DMAs before any matmul runs
            q0h0[0](); k0h0[0]()
            for th in q0h0[1:]:
                th()
            for th in k0h0[1:]:
                th()
            q0 = proj_thunks(0, qT_d, wq_tiles, qTs[0], bq_sb, halves=(1,))
            k0 = proj_thunks(0, kT_d, wk_tiles, kTs[0], bk_sb, halves=(1,))
            wv_tiles = load_wtiles(WvT_d)

            # per-stage extra work, placed just-in-time:
            #  stage 0: K0/Q0 second halves + V-projection first s-half
            #  stage 1: V-projection second s-half
            #  pair p>=1: K-half0 @4p-2, Q-half0 @4p-1, K-half1 @4p, Q-half1 @4p+1
            #  stages 14, 15: out-proj for t0, t1
            vpA = vproj_thunks(wv_tiles, 0)
            extras = {0: k0 + q0 + vpA[:10], 1: vpA[10:]}
            for p in range(1, NP):
                qp = proj_thunks(p, qT_d, wq_tiles, qTs[p], bq_sb)
                kp = proj_thunks(p, kT_d, wk_tiles, kTs[p], bk_sb)
                for sg, th in ((4 * p - 2, kp[:10]), (4 * p - 1, qp[:10]),
                               (4 * p, kp[10:]), (4 * p + 1, qp[10:])):
                    extras[sg] = extras.get(sg, []) + th
            # second head-quad of V, appended after each stage's proj work
            # (sequential mx-slot handoff, done well before stage 9 needs
            # it). Skip stages 4/5 whose proj work has a hard slot-8
            # deadline (dl=7) — vpB has no deadline and shouldn't compete.
            vpB = vproj_thunks(wv_tiles, 1)
            n4 = (len(vpB) + 3) // 4
            for i, sg in enumerate((2, 3, 6, 7)):
                extras[sg] = extras.get(sg, []) + vpB[i * n4:(i + 1) * n4]
            extras[14] = extras.get(14, []) + outproj_thunks(0)
            extras[15] = extras.get(15, []) + outproj_thunks(1)

            # pacing deadlines: K-half1 stages (4p) must finish extras by
            # slot 8 (their own scores need those kT columns); stages 0/1
            # feed vaug just-in-time; elsewhere spread smoothly. Stages
            # 14/15's out-proj starts at slot 2 (their Onorm quarter lands
            # via the DMA transpose issued at the previous stage's end).
            dls = {0: 14, 1: 14, 4: 7, 8: 7, 12: 7, 14: 14, 15: 14}
            sl0s = {14: 2, 15: 2}
            prev = None
            for s in range(16):
                p, tq = s // 4, s % 4
                prev = emit_stage(p, tq, prev, extras.get(s, []),
                                  dl=dls.get(s, 16), sl0=sl0s.get(s, 0))

            # tail: PV of the last stage with out-proj(t2) interleaved
            # (its Onorm slices land with the transpose at the end of stage
            # 15), then the final normalize+transpose and out-proj(t3)
            op2 = outproj_thunks(2)
            taken = 0
            for st in range(NS):
                if st >= 2:
                    want = (len(op2) * (st - 1) + 13) // 14
                    while taken < want:
                        op2[taken]()
                        taken += 1
                emit_pv_slot(prev, st)
            emit_normalize(prev)
            outproj_tail(3)

    nc.compile()
    return nc


def _get_nc():
    global _cached
    if _cached is None:
        _cached = _build()
    return _cached


def _prep_core_inputs(c, query, key, value, Wq, Wk, Wv, Wo, bq, bk, bv,
                      _cache={}):
    b, g = c // 2, c % 2
    sl = slice(g * DC, (g + 1) * DC)
    key_ = (id(query), b)
    if key_ not in _cache:
        # both cores of a batch share the transposed/cast activations
        _cache.clear()
        _cache[key_] = {
            "qT": query[b].T.astype(_BF16),
            "kT": key[b].T.astype(_BF16),
            "vT": value[b].T.astype(_BF16),
        }
    shared = _cache[key_]
    return {
        **shared,
        "WqT": Wq[sl].T.astype(_BF16),
        "WkT": Wk[sl].T.astype(_BF16),
        "WvT": Wv[sl].T.astype(_BF16),
        "WoS": Wo[:, sl].T.astype(_BF16),
        "bq": np.ascontiguousarray(bq[sl].reshape(NP, 128).T),
        "bk": np.ascontiguousarray(bk[sl].reshape(NP, 128).T),
        "bv": np.ascontiguousarray(bv[sl].reshape(1, DC)),
    }


def kernel(**inputs):
    from concourse.bass_utils import run_bass_kernel_spmd

    args = {k: np.asarray(inputs[k], np.float32)
            for k in ("query", "key", "value", "Wq", "Wk", "Wv", "Wo",
                      "bq", "bk", "bv", "bo")}
    _prep_core_inputs.__defaults__[0].clear()
    nc = _get_nc()
    in_maps = [
        _prep_core_inputs(c, args["query"], args["key"], args["value"],
                          args["Wq"], args["Wk"], args["Wv"], args["Wo"],
                          args["bq"], args["bk"], args["bv"])
        for c in range(8)
    ]
    res = run_bass_kernel_spmd(nc, in_maps, core_ids=list(range(8)))
    outs = [r["out"] for r in res.results]
    final = np.empty((B, T, E), np.float32)
    for b in range(B):
        final[b] = outs[2 * b] + outs[2 * b + 1] + args["bo"][None, :]
    return final


# revision 6
# speedup vs baseline: 1.1410x; 1.0621x over previous
"""Multi-head attention (B=4, T=S=2048, E=1024, H=16, D=64) on 8 TRN2 NeuronCores.

Sharding: core c handles batch b=c//2 and head-group g=c%2 (8 of 16 heads).
Each core computes its 8 heads' attention plus the matching column-slice of
the output projection, producing a partial [T, E] f32 output. Host sums the
two partials per batch and adds bo.

On-chip dataflow (all matmuls bf16 with fp32 PSUM accumulation):
  qT[d,t] = WqT.T @ queryT       (d-major projections, per 128-dim head pair)
  kT[d,t] likewise; v[s,d] natural via value.T as the stationary operand
  S.T[s,t] = kT_h.T @ qT_h       (two heads row-packed in the 128-row PE array)
  expS.T   = exp(S.T * 1/8)      (ScalarE, PSUM -> SBUF bf16)
  [O;den]  = expS_blk.T @ [v_h|1]  (exp block stationary, narrow 65-col v
                                    moving -> O in [t,d] layout + per-t dens)
  Onorm_td = O * (1/den)         (den is per-PARTITION here: one cheap DVE
                                  tensor_tensor, no cross-partition bcast)
  OnormT   = XBAR DMA-transpose of Onorm_td -> d-major   (no PE/PSUM cost)
  partial  = OnormT.T @ WoSlice  (accumulate over the core's 4 head pairs)

The [t,d]-output PV orientation keeps the PE's moving operand narrow (65
cols vs 512), halving PV time; denominators ride along as a 65th moving
column of ones, and the d-major layout the out-projection needs is restored
by the DMA crossbar instead of the PE.

Inputs stream in few, large DMAs (one per weight tensor, two per
projection half) issued a stage ahead of their consumers; projections are
emitted as quarter-granular accumulation groups so their PSUM residency is
short and their PE work spreads smoothly across the score slots.

Emission is software-pipelined: stage s=(pair, t-quarter) in pair-major
order; each stage's 16 score-tile slots interleave the previous stage's PV
accumulation plus spread-out projection / v-projection / out-projection
work, keeping ScalarE (the exp bottleneck) continuously fed.
"""

from contextlib import ExitStack

import numpy as np
import ml_dtypes

B, T, S, E = 4, 2048, 2048, 1024
H, D = 16, 64
DC = 512          # dims per core (8 heads x 64)
NP = 4            # head pairs per core
NS = S // 128     # 16 s-tiles
NQ = 4            # t-quarters of 512

_BF16 = ml_dtypes.bfloat16

_cached = None


def _build(repeats=1):
    import concourse.bass as bass
    import concourse.mybir as mybir
    import concourse.tile as tile
    from concourse import bacc

    f32 = mybir.dt.float32
    bf16 = mybir.dt.bfloat16
    AF = mybir.ActivationFunctionType

    nc = bacc.Bacc("TRN2", target_bir_lowering=False)

    qT_d = nc.dram_tensor("qT", [E, T], bf16, kind="ExternalInput")
    kT_d = nc.dram_tensor("kT", [E, S], bf16, kind="ExternalInput")
    vT_d = nc.dram_tensor("vT", [E, S], bf16, kind="ExternalInput")
    WqT_d = nc.dram_tensor("WqT", [E, DC], bf16, kind="ExternalInput")
    WkT_d = nc.dram_tensor("WkT", [E, DC], bf16, kind="ExternalInput")
    WvT_d = nc.dram_tensor("WvT", [E, DC], bf16, kind="ExternalInput")
    WoS_d = nc.dram_tensor("WoS", [DC, E], bf16, kind="ExternalInput")
    bq_d = nc.dram_tensor("bq", [128, NP], f32, kind="ExternalInput")
    bk_d = nc.dram_tensor("bk", [128, NP], f32, kind="ExternalInput")
    bv_d = nc.dram_tensor("bv", [1, DC], f32, kind="ExternalInput")
    out_d = nc.dram_tensor("out", [T, E], f32, kind="ExternalOutput")

    with tile.TileContext(nc) as tc, ExitStack() as ctx:
        persist = ctx.enter_context(tc.tile_pool(name="persist", bufs=1))
        psc = ctx.enter_context(tc.tile_pool(name="psc", bufs=2, space="PSUM"))
        ppv = ctx.enter_context(tc.tile_pool(name="ppv", bufs=1, space="PSUM"))
        pmx = ctx.enter_context(tc.tile_pool(name="pmx", bufs=2, space="PSUM"))
        expool = ctx.enter_context(tc.tile_pool(name="expool", bufs=20))
        small = ctx.enter_context(tc.tile_pool(name="small", bufs=2))
        ocp_pool = ctx.enter_context(tc.tile_pool(name="ocp", bufs=3))
        xin = ctx.enter_context(tc.tile_pool(name="xin", bufs=6))
        wpool = ctx.enter_context(tc.tile_pool(name="wts", bufs=3))

        # ---- persistent SBUF tiles ----
        qTs = [persist.tile([128, T], bf16, tag=f"qT{p}", name=f"qT{p}") for p in range(NP)]
        kTs = [persist.tile([128, S], bf16, tag=f"kT{p}", name=f"kT{p}") for p in range(NP)]
        vaug = [persist.tile([128, 8 * 65], bf16, tag=f"va{st}", name=f"va{st}") for st in range(NS)]
        WoSs = [persist.tile([128, E], bf16, tag=f"wo{p}", name=f"wo{p}") for p in range(NP)]
        Onorm = [persist.tile([128, T], bf16, tag=f"on{p}", name=f"on{p}") for p in range(NP)]
        bq_sb = persist.tile([128, NP], f32, tag="bq", name="bq_sb")
        bk_sb = persist.tile([128, NP], f32, tag="bk", name="bk_sb")
        bv_sb = persist.tile([128, DC], f32, tag="bv", name="bv_sb")

        nc.sync.dma_start(out=bq_sb, in_=bq_d[:, :])
        nc.sync.dma_start(out=bk_sb, in_=bk_d[:, :])
        bv_ap = bv_d[:, :]
        bv_bcast_ap = bass.AP(
            tensor=bv_ap.tensor,
            offset=bv_ap.offset,
            ap=[[0, 128], bv_ap.ap[-1]],
        )
        nc.sync.dma_start(out=bv_sb, in_=bv_bcast_ap)
        for st in range(NS):
            va3 = vaug[st].rearrange("p (h x) -> p h x", x=65)
            nc.vector.memset(va3[:, :, 64:65], 1.0)

        def load_w(dram):
            """One [128, 8x512] tile holding all 8 contraction chunks of a
            projection weight, loaded in a single DMA; chunk e's stationary
            block for pair p sits at cols e*512 + p*128."""
            wt = wpool.tile([128, 8 * DC], bf16, tag="w", name="wt")
            nc.sync.dma_start(
                out=wt.rearrange("p (c d) -> p c d", d=DC),
                in_=dram[:, :].rearrange("(c p) d -> p c d", p=128),
            )
            return wt

        def xhalf_dma(x_dram, half):
            """Two [128, 4x1024] tiles covering the 8 contraction chunks of
            one 1024-col half of an activation input (two DMAs)."""
            xh = []
            for ci in range(2):
                xt = xin.tile([128, 4096], bf16, tag="xin", name="xin")
                nc.sync.dma_start(
                    out=xt.rearrange("p (c t) -> p c t", t=1024),
                    in_=x_dram[ci * 512:(ci + 1) * 512,
                               half * 1024:(half + 1) * 1024].rearrange(
                                   "(c p) t -> p c t", p=128),
                )
                xh.append(xt)
            return xh

        def dma_box(x_dram, half):
            """Thunk that issues xhalf_dma when called (placed a stage ahead
            of the matmul consumers), exposing the tiles via .get()."""
            box = []
            def run():
                box.extend(xhalf_dma(x_dram, half))
            run.get = lambda: box
            return run

        def proj_quarters(p, wt, get_xh, dst, bias_sb, half):
            """Work thunks for one projection half: two quarter-granular
            accumulation groups (alloc+4 MMs, then 4 MMs+bias each). get_xh
            resolves the input tiles at emission time (they were DMA'd a
            stage earlier)."""
            thunks = []
            for qi in range(2):
                q = half * 2 + qi
                ps_box = []

                def mm_lo(qi=qi, ps_box=ps_box):
                    xh = get_xh()
                    ps_box.append(pmx.tile([128, 512], f32, tag="mx", name="mx_ps"))
                    for e in range(4):
                        nc.tensor.matmul(
                            ps_box[0],
                            wt[:, e * 512 + p * 128:e * 512 + (p + 1) * 128],
                            xh[0][:, e * 1024 + qi * 512:e * 1024 + qi * 512 + 512],
                            start=(e == 0), stop=False,
                        )

                def mm_hi(q=q, qi=qi, ps_box=ps_box):
                    xh = get_xh()
                    for e in range(4):
                        nc.tensor.matmul(
                            ps_box[0],
                            wt[:, (e + 4) * 512 + p * 128:(e + 4) * 512 + (p + 1) * 128],
                            xh[1][:, e * 1024 + qi * 512:e * 1024 + qi * 512 + 512],
                            start=False, stop=(e == 3),
                        )
                    nc.vector.tensor_scalar_add(
                        dst[:, q * 512:(q + 1) * 512],
                        ps_box[0], bias_sb[:, p:p + 1])
                    ps_box.clear()

                thunks += [mm_lo, mm_hi]
            return thunks

        def vproj_units(wt, dh, half, get_vh, stis=range(8)):
            """V-projection work for head-quad dh over one s-half: per unit
            the full-E contraction for one 128-row s-tile + bias into the
            augmented-v layout."""
            thunks = []
            for sti in stis:
                def vst(sti=sti, half=half):
                    vh = get_vh()
                    st = half * 8 + sti
                    ps = pmx.tile([128, 512], f32, tag="mx", name="mx_ps")
                    for e in range(8):
                        nc.tensor.matmul(
                            ps[:, 0:256],
                            vh[e // 4][:, (e % 4) * 1024 + sti * 128:
                                       (e % 4) * 1024 + (sti + 1) * 128],
                            wt[:, e * 512 + dh * 256:e * 512 + (dh + 1) * 256],
                            start=(e == 0),
                            stop=(e == 7),
                        )
                    va3 = vaug[st].rearrange("p (h x) -> p h x", x=65)
                    nc.vector.tensor_add(
                        va3[:, dh * 4:(dh + 1) * 4, 0:64],
                        ps[:, 0:256].rearrange("p (h x) -> p h x", x=64),
                        bv_sb[:, dh * 256:(dh + 1) * 256].rearrange(
                            "p (h x) -> p h x", x=64),
                    )
                thunks.append(vst)
            return thunks

        def outproj_thunks(tq):
            thunks = []
            for tt in range(tq * 4, tq * 4 + 4):
                for c in range(2):
                    def unit(tt=tt, c=c):
                        op_ps = pmx.tile([128, 512], f32, tag="mx", name="mx_ps")
                        for p in range(NP):
                            nc.tensor.matmul(
                                op_ps,
                                Onorm[p][:, tt * 128:(tt + 1) * 128],
                                WoSs[p][:, c * 512:(c + 1) * 512],
                                start=(p == 0),
                                stop=(p == 3),
                            )
                        oc = ocp_pool.tile([128, 512], f32, tag="ocp", name="oc")
                        nc.vector.tensor_copy(oc, op_ps)
                        nc.sync.dma_start(
                            out=out_d[tt * 128:(tt + 1) * 128,
                                      c * 512:(c + 1) * 512],
                            in_=oc)
                    thunks.append(unit)
            return thunks

        def outproj_tail(tq):
            # tail variant: "sc" psum tiles are free once scoring has ended,
            # so use wide [128,1024] units to avoid mx-slot serialization
            for tt in range(tq * 4, tq * 4 + 4):
                op_ps = psc.tile([128, 1024], f32, tag="sc", name="sc_ps")
                for c in range(2):
                    for p in range(NP):
                        nc.tensor.matmul(
                            op_ps[:, c * 512:(c + 1) * 512],
                            Onorm[p][:, tt * 128:(tt + 1) * 128],
                            WoSs[p][:, c * 512:(c + 1) * 512],
                            start=(p == 0),
                            stop=(p == 3),
                        )
                oc = ocp_pool.tile([128, 1024], f32, tag="ocpw", name="ocw")
                nc.vector.tensor_copy(oc, op_ps)
                nc.sync.dma_start(out=out_d[tt * 128:(tt + 1) * 128, :], in_=oc)

        class PrevStage:
            def __init__(self, p, tq, exs):
                self.p, self.tq, self.exs = p, tq, exs
                self.pv = None      # two [128, 4*65] psum tiles (4 units each)

        def emit_pv_slot(prev, st):
            """8 accumulating matmuls: O_td[t, d]/den for each (tb, h) unit,
            with exp (s x t block) stationary and the 65-col augmented v
            moving. Unit g = tb*2 + h lives at cols (g%4)*65 of pv tile g//4;
            one start per PSUM bank (unit 0 of each tile, which pending-zeroes
            the whole bank), one stop (unit 3)."""
            if st == 0:
                prev.pv = [
                    ppv.tile([128, 260], f32, tag="pvA", name="pvA",
                             padded_shape=[128, 512]),
                    ppv.tile([128, 260], f32, tag="pvB", name="pvB",
                             padded_shape=[128, 512]),
                ]
            for g in range(8):
                tb, h = g // 2, g % 2
                pvt = prev.pv[g // 4]
                u = g % 4
                hidx = 2 * prev.p + h
                nc.tensor.matmul(
                    pvt[:, u * 65:(u + 1) * 65],
                    prev.exs[st][:, h * 512 + tb * 128:h * 512 + (tb + 1) * 128],
                    vaug[st][:, hidx * 65:hidx * 65 + 65],
                    start=(st == 0 and u == 0),
                    stop=(st == 15 and u == 3),
                )

        def emit_normalize(prev):
            """den sits on the same partition as its O row: reciprocal of the
            65th column of each unit, one tensor_tensor mul per pv tile (rc
            broadcast along d via a stride-0 AP) into a [t, d] bf16 tile, then
            one XBAR DMA transpose restoring the d-major layout out-proj
            wants (out 3D => per-128-block transpose)."""
            rc = small.tile([128, 8], f32, tag="rc", name="rc")
            otd = small.tile([128, 512], bf16, tag="otd", name="otd")
            for i in range(2):
                pv3 = prev.pv[i].rearrange("p (u x) -> p u x", x=65)
                nc.vector.reciprocal(rc[:, i * 4:(i + 1) * 4], pv3[:, :, 64:65])
            for i in range(2):
                pv3 = prev.pv[i].rearrange("p (u x) -> p u x", x=65)
                rc_b = bass.AP(
                    tensor=rc.tensor,
                    offset=rc[:, i * 4:(i + 1) * 4].offset,
                    ap=[rc.ap[0], [1, 4], [0, 64]],
                )
                nc.vector.tensor_mul(
                    otd[:, i * 256:(i + 1) * 256].rearrange(
                        "p (u x) -> p u x", x=64),
                    pv3[:, :, 0:64],
                    rc_b,
                )
            t0 = prev.tq * 512
            nc.sync.dma_start_transpose(
                out=Onorm[prev.p][:, t0:t0 + 512].rearrange(
                    "p (b t) -> p b t", t=128),
                in_=otd,
            )
            prev.pv = None

        def emit_stage(p, tq, prev, extras, dl=16, sl0=0):
            """16 score slots for (p, tq); interleave prev stage's PV and
            the extra thunks (emitted between slots sl0..sl0+dl); returns
            this stage's record."""
            t0 = tq * 512
            exs = []
            n_ex = len(extras)
            taken = 0
            for st in range(NS):
                sc_ps = psc.tile([128, 1024], f32, tag="sc", name="sc_ps")
                nc.tensor.matmul(
                    sc_ps[:, 0:512],
                    kTs[p][0:64, st * 128:(st + 1) * 128],
                    qTs[p][0:64, t0:t0 + 512],
                    start=True, stop=True,
                    tile_position=(0, 0),
                )
                nc.tensor.matmul(
                    sc_ps[:, 512:1024],
                    kTs[p][64:128, st * 128:(st + 1) * 128],
                    qTs[p][64:128, t0:t0 + 512],
                    start=True, stop=True,
                    tile_position=(64, 0),
                )
                ex = expool.tile([128, 1024], bf16, tag="ex", name="ex")
                nc.scalar.activation(ex, sc_ps, AF.Exp, scale=0.125)
                exs.append(ex)
                if st >= sl0:
                    prog = min(st - sl0 + 1, dl)
                    want = (n_ex * prog + dl - 1) // dl
                    while taken < want:
                        extras[taken]()
                        taken += 1
                if prev is not None:
                    emit_pv_slot(prev, st)
            while taken < n_ex:
                extras[taken]()
                taken += 1
            if prev is not None:
                emit_normalize(prev)
            return PrevStage(p, tq, exs)

        # ---- emission ----
        for _rep in range(repeats):
            # startup: stream in pair-0's q/k projections; the first scores
            # need q0 half0 and k0 half0 (slots 0-7).
            wq = load_w(WqT_d)
            xq0 = xhalf_dma(qT_d, 0)
            wk = load_w(WkT_d)
            xk0 = xhalf_dma(kT_d, 0)
            for th in proj_quarters(0, wq, lambda: xq0, qTs[0], bq_sb, 0):
                th()
            xk1 = xhalf_dma(kT_d, 1)           # k0 half1: needed stage-0 slot 8
            for th in proj_quarters(0, wk, lambda: xk0, kTs[0], bk_sb, 0):
                th()
            wv = load_w(WvT_d)
            xv0 = xhalf_dma(vT_d, 0)           # vpA h0: vaug st0-7 for stage-1 PV
            xq1 = xhalf_dma(qT_d, 1)           # q0 half1: needed stage 2
            for p in range(NP):
                nc.sync.dma_start(out=WoSs[p], in_=WoS_d[p * 128:(p + 1) * 128, :])

            extras = {}

            def add(sg, ths):
                extras[sg] = extras.get(sg, []) + ths

            # stage 0: k0 half1 (scores slots 8-15 of this very stage), the
            # v-projection's first s-half, and the DMA for its second
            xv1_t = dma_box(vT_d, 1)
            add(0, proj_quarters(0, wk, lambda: xk1, kTs[0], bk_sb, 1))
            add(0, [xv1_t])
            add(0, vproj_units(wv, 0, 0, lambda: xv0))
            # stage 1: q0 half1 + v-projection second s-half (vaug st8-15,
            # JIT for this stage's own PV slots)
            add(1, proj_quarters(0, wq, lambda: xq1, qTs[0], bq_sb, 1))
            add(1, vproj_units(wv, 0, 1, xv1_t.get))

            # pairs 1-3 q/k: work at stages 4p-2 (k h0), 4p-1 (q h0), 4p
            # (k h1, hard slot-8 deadline), 4p+1 (q h1); each half's two
            # DMAs issue one stage ahead.
            for p in range(1, NP):
                tk0 = dma_box(kT_d, 0)
                tq0 = dma_box(qT_d, 0)
                tk1 = dma_box(kT_d, 1)
                tq1 = dma_box(qT_d, 1)
                add(4 * p - 3, [tk0])
                add(4 * p - 2, proj_quarters(p, wk, tk0.get, kTs[p], bk_sb, 0) + [tq0])
                add(4 * p - 1, proj_quarters(p, wq, tq0.get, qTs[p], bq_sb, 0) + [tk1])
                add(4 * p, proj_quarters(p, wk, tk1.get, kTs[p], bk_sb, 1) + [tq1])
                add(4 * p + 1, proj_quarters(p, wq, tq1.get, qTs[p], bq_sb, 1))

            # second head-quad of V (pairs 2-3, needed from stage 9's PV):
            # spread over lighter stages 2/3/5/6, DMAs one stage ahead.
            xvB0_t = dma_box(vT_d, 0)
            xvB1_t = dma_box(vT_d, 1)
            add(1, [xvB0_t])
            add(2, vproj_units(wv, 1, 0, xvB0_t.get, range(0, 4)))
            add(3, vproj_units(wv, 1, 0, xvB0_t.get, range(4, 8)))
            add(4, [xvB1_t])
            add(5, vproj_units(wv, 1, 1, xvB1_t.get, range(0, 4)))
            add(6, vproj_units(wv, 1, 1, xvB1_t.get, range(4, 8)))

            add(14, outproj_thunks(0))
            add(15, outproj_thunks(1))

            # pacing deadlines: K-half1 stages (4p) must finish extras by
            # slot 8 (their own scores need those kT columns); stages 0/1
            # feed vaug just-in-time; elsewhere spread smoothly. Stages
            # 14/15's out-proj starts at slot 2 (their Onorm quarter lands
            # via the DMA transpose issued at the previous stage's end).
            dls = {0: 14, 1: 14, 4: 8, 8: 8, 12: 8, 14: 14, 15: 14}
            sl0s = {14: 2, 15: 2}
            prev = None
            for s in range(16):
                p, tq = s // 4, s % 4
                prev = emit_stage(p, tq, prev, extras.get(s, []),
                                  dl=dls.get(s, 16), sl0=sl0s.get(s, 0))

            # tail: PV of the last stage with out-proj(t2) interleaved
            # (its Onorm slices land with the transpose at the end of stage
            # 15), then the final normalize+transpose and out-proj(t3)
            op2 = outproj_thunks(2)
            taken = 0
            for st in range(NS):
                if st >= 2:
                    want = (len(op2) * (st - 1) + 13) // 14
                    while taken < want:
                        op2[taken]()
                        taken += 1
                emit_pv_slot(prev, st)
            emit_normalize(prev)
            outproj_tail(3)

    nc.compile()
    return nc


def _get_nc():
    global _cached
    if _cached is None:
        _cached = _build()
    return _cached


def _prep_core_inputs(c, query, key, value, Wq, Wk, Wv, Wo, bq, bk, bv,
                      _cache={}):
    b, g = c // 2, c % 2
    sl = slice(g * DC, (g + 1) * DC)
    key_ = (id(query), b)
    if key_ not in _cache:
        # both cores of a batch share the transposed/cast activations
        _cache.clear()
        _cache[key_] = {
            "qT": query[b].T.astype(_BF16),
            "kT": key[b].T.astype(_BF16),
            "vT": value[b].T.astype(_BF16),
        }
    shared = _cache[key_]
    return {
        **shared,
        "WqT": Wq[sl].T.astype(_BF16),
        "WkT": Wk[sl].T.astype(_BF16),
        "WvT": Wv[sl].T.astype(_BF16),
        "WoS": Wo[:, sl].T.astype(_BF16),
        "bq": np.ascontiguousarray(bq[sl].reshape(NP, 128).T),
        "bk": np.ascontiguousarray(bk[sl].reshape(NP, 128).T),
        "bv": np.ascontiguousarray(bv[sl].reshape(1, DC)),
    }


def kernel(**inputs):
    from concourse.bass_utils import run_bass_kernel_spmd

    args = {k: np.asarray(inputs[k], np.float32)
            for k in ("query", "key", "value", "Wq", "Wk", "Wv", "Wo",
                      "bq", "bk", "bv", "bo")}
    _prep_core_inputs.__defaults__[0].clear()
    nc = _get_nc()
    in_maps = [
        _prep_core_inputs(c, args["query"], args["key"], args["value"],
                          args["Wq"], args["Wk"], args["Wv"], args["Wo"],
                          args["bq"], args["bk"], args["bv"])
        for c in range(8)
    ]
    res = run_bass_kernel_spmd(nc, in_maps, core_ids=list(range(8)))
    outs = [r["out"] for r in res.results]
    final = np.empty((B, T, E), np.float32)
    for b in range(B):
        final[b] = outs[2 * b] + outs[2 * b + 1] + args["bo"][None, :]
    return final


# revision 10
# speedup vs baseline: 1.1443x; 1.0029x over previous
"""Multi-head attention (B=4, T=S=2048, E=1024, H=16, D=64) on 8 TRN2 NeuronCores.

Sharding: core c handles batch b=c//2 and head-group g=c%2 (8 of 16 heads).
Each core computes its 8 heads' attention plus the matching column-slice of
the output projection, producing a partial [T, E] f32 output. Host sums the
two partials per batch and adds bo.

On-chip dataflow (all matmuls bf16 with fp32 PSUM accumulation):
  qT[d,t] = WqT.T @ queryT       (d-major projections, per 128-dim head pair)
  kT[d,t] likewise; v[s,d] natural via value.T as the stationary operand
  S.T[s,t] = kT_h.T @ qT_h       (two heads row-packed in the 128-row PE array)
  expS.T   = exp(S.T * 1/8)      (ScalarE, PSUM -> SBUF bf16)
  [O;den]  = expS_blk.T @ [v_h|1]  (exp block stationary, narrow 65-col v
                                    moving -> O in [t,d] layout + per-t dens)
  Onorm_td = O * (1/den)         (den is per-PARTITION here: one cheap DVE
                                  tensor_tensor, no cross-partition bcast)
  OnormT   = XBAR DMA-transpose of Onorm_td -> d-major   (no PE/PSUM cost)
  partial  = OnormT.T @ WoSlice  (accumulate over the core's 4 head pairs)

The [t,d]-output PV orientation keeps the PE's moving operand narrow (65
cols vs 512), halving PV time; denominators ride along as a 65th moving
column of ones, and the d-major layout the out-projection needs is restored
by the DMA crossbar instead of the PE.

Inputs stream in few, large DMAs (one per weight tensor, two per
projection half) issued a stage ahead of their consumers; projections are
emitted as quarter-granular accumulation groups so their PSUM residency is
short and their PE work spreads smoothly across the score slots.

Emission is software-pipelined: stage s=(pair, t-quarter) in pair-major
order; each stage's 16 score-tile slots interleave the previous stage's PV
accumulation plus spread-out projection / v-projection / out-projection
work, keeping ScalarE (the exp bottleneck) continuously fed.
"""

from contextlib import ExitStack

import numpy as np
import ml_dtypes

B, T, S, E = 4, 2048, 2048, 1024
H, D = 16, 64
DC = 512          # dims per core (8 heads x 64)
NP = 4            # head pairs per core
NS = S // 128     # 16 s-tiles
NQ = 4            # t-quarters of 512

_BF16 = ml_dtypes.bfloat16

_cached = None


def _build(repeats=1):
    import concourse.bass as bass
    import concourse.mybir as mybir
    import concourse.tile as tile
    from concourse import bacc

    f32 = mybir.dt.float32
    bf16 = mybir.dt.bfloat16
    AF = mybir.ActivationFunctionType

    nc = bacc.Bacc("TRN2", target_bir_lowering=False)

    qT_d = nc.dram_tensor("qT", [E, T], bf16, kind="ExternalInput")
    kT_d = nc.dram_tensor("kT", [E, S], bf16, kind="ExternalInput")
    vT_d = nc.dram_tensor("vT", [E, S], bf16, kind="ExternalInput")
    WqT_d = nc.dram_tensor("WqT", [E, DC], bf16, kind="ExternalInput")
    WkT_d = nc.dram_tensor("WkT", [E, DC], bf16, kind="ExternalInput")
    WvT_d = nc.dram_tensor("WvT", [E, DC], bf16, kind="ExternalInput")
    WoS_d = nc.dram_tensor("WoS", [DC, E], bf16, kind="ExternalInput")
    bq_d = nc.dram_tensor("bq", [128, NP], f32, kind="ExternalInput")
    bk_d = nc.dram_tensor("bk", [128, NP], f32, kind="ExternalInput")
    bv_d = nc.dram_tensor("bv", [1, DC], f32, kind="ExternalInput")
    out_d = nc.dram_tensor("out", [T, E], f32, kind="ExternalOutput")

    with tile.TileContext(nc) as tc, ExitStack() as ctx:
        persist = ctx.enter_context(tc.tile_pool(name="persist", bufs=1))
        psc = ctx.enter_context(tc.tile_pool(name="psc", bufs=2, space="PSUM"))
        ppv = ctx.enter_context(tc.tile_pool(name="ppv", bufs=1, space="PSUM"))
        pmx = ctx.enter_context(tc.tile_pool(name="pmx", bufs=2, space="PSUM"))
        expool = ctx.enter_context(tc.tile_pool(name="expool", bufs=20))
        small = ctx.enter_context(tc.tile_pool(name="small", bufs=2))
        ocp_pool = ctx.enter_context(tc.tile_pool(name="ocp", bufs=3))
        xin = ctx.enter_context(tc.tile_pool(name="xin", bufs=6))
        wpool = ctx.enter_context(tc.tile_pool(name="wts", bufs=3))

        # ---- persistent SBUF tiles ----
        qTs = [persist.tile([128, T], bf16, tag=f"qT{p}", name=f"qT{p}") for p in range(NP)]
        kTs = [persist.tile([128, S], bf16, tag=f"kT{p}", name=f"kT{p}") for p in range(NP)]
        vaug = [persist.tile([128, 8 * 65], bf16, tag=f"va{st}", name=f"va{st}") for st in range(NS)]
        WoSs = [persist.tile([128, E], bf16, tag=f"wo{p}", name=f"wo{p}") for p in range(NP)]
        Onorm = [persist.tile([128, T], bf16, tag=f"on{p}", name=f"on{p}") for p in range(NP)]
        bq_sb = persist.tile([128, NP], f32, tag="bq", name="bq_sb")
        bk_sb = persist.tile([128, NP], f32, tag="bk", name="bk_sb")
        bv_sb = persist.tile([128, DC], f32, tag="bv", name="bv_sb")

        nc.sync.dma_start(out=bq_sb, in_=bq_d[:, :])
        nc.sync.dma_start(out=bk_sb, in_=bk_d[:, :])
        bv_ap = bv_d[:, :]
        bv_bcast_ap = bass.AP(
            tensor=bv_ap.tensor,
            offset=bv_ap.offset,
            ap=[[0, 128], bv_ap.ap[-1]],
        )
        nc.sync.dma_start(out=bv_sb, in_=bv_bcast_ap)
        for st in range(NS):
            va3 = vaug[st].rearrange("p (h x) -> p h x", x=65)
            nc.vector.memset(va3[:, :, 64:65], 1.0)

        def load_w(dram, split=False):
            """One [128, 8x512] tile holding all 8 contraction chunks of a
            projection weight; chunk e's stationary block for pair p sits at
            cols e*512 + p*128. With split=True, pair-0's slices (the only
            ones the startup stages touch) load first in a small DMA and a
            thunk for the remaining pairs is returned."""
            wt = wpool.tile([128, 8 * DC], bf16, tag="w", name="wt")
            w3 = wt.rearrange("p (c d) -> p c d", d=DC)
            s3 = dram[:, :].rearrange("(c p) d -> p c d", p=128)
            if not split:
                nc.sync.dma_start(out=w3, in_=s3)
                return wt
            nc.sync.dma_start(out=w3[:, :, 0:128], in_=s3[:, :, 0:128])
            def rest():
                nc.sync.dma_start(out=w3[:, :, 128:DC], in_=s3[:, :, 128:DC])
            return wt, rest

        def xhalf_dma(x_dram, half):
            """Two [128, 4x1024] tiles covering the 8 contraction chunks of
            one 1024-col half of an activation input (two DMAs)."""
            xh = []
            for ci in range(2):
                xt = xin.tile([128, 4096], bf16, tag="xin", name="xin")
                nc.sync.dma_start(
                    out=xt.rearrange("p (c t) -> p c t", t=1024),
                    in_=x_dram[ci * 512:(ci + 1) * 512,
                               half * 1024:(half + 1) * 1024].rearrange(
                                   "(c p) t -> p c t", p=128),
                )
                xh.append(xt)
            return xh

        def dma_box(x_dram, half):
            """Thunk that issues xhalf_dma when called (placed a stage ahead
            of the matmul consumers), exposing the tiles via .get()."""
            box = []
            def run():
                box.extend(xhalf_dma(x_dram, half))
            run.get = lambda: box
            return run

        def proj_quarters(p, wt, get_xh, dst, bias_sb, half):
            """Work thunks for one projection half: two quarter-granular
            accumulation groups (alloc+4 MMs, then 4 MMs+bias each). get_xh
            resolves the input tiles at emission time (they were DMA'd a
            stage earlier)."""
            thunks = []
            for qi in range(2):
                q = half * 2 + qi
                ps_box = []

                def mm_lo(qi=qi, ps_box=ps_box):
                    xh = get_xh()
                    ps_box.append(pmx.tile([128, 512], f32, tag="mx", name="mx_ps"))
                    for e in range(4):
                        nc.tensor.matmul(
                            ps_box[0],
                            wt[:, e * 512 + p * 128:e * 512 + (p + 1) * 128],
                            xh[0][:, e * 1024 + qi * 512:e * 1024 + qi * 512 + 512],
                            start=(e == 0), stop=False,
                        )

                def mm_hi(q=q, qi=qi, ps_box=ps_box):
                    xh = get_xh()
                    for e in range(4):
                        nc.tensor.matmul(
                            ps_box[0],
                            wt[:, (e + 4) * 512 + p * 128:(e + 4) * 512 + (p + 1) * 128],
                            xh[1][:, e * 1024 + qi * 512:e * 1024 + qi * 512 + 512],
                            start=False, stop=(e == 3),
                        )
                    nc.vector.tensor_scalar_add(
                        dst[:, q * 512:(q + 1) * 512],
                        ps_box[0], bias_sb[:, p:p + 1])
                    ps_box.clear()

                thunks += [mm_lo, mm_hi]
            return thunks

        def vproj_units(wt, dh, half, get_vh, stis=range(8)):
            """V-projection work for head-quad dh over one s-half: per unit
            the full-E contraction for one 128-row s-tile + bias into the
            augmented-v layout."""
            thunks = []
            for sti in stis:
                def vst(sti=sti, half=half):
                    vh = get_vh()
                    st = half * 8 + sti
                    ps = pmx.tile([128, 512], f32, tag="mx", name="mx_ps")
                    for e in range(8):
                        nc.tensor.matmul(
                            ps[:, 0:256],
                            vh[e // 4][:, (e % 4) * 1024 + sti * 128:
                                       (e % 4) * 1024 + (sti + 1) * 128],
                            wt[:, e * 512 + dh * 256:e * 512 + (dh + 1) * 256],
                            start=(e == 0),
                            stop=(e == 7),
                        )
                    va3 = vaug[st].rearrange("p (h x) -> p h x", x=65)
                    nc.vector.tensor_add(
                        va3[:, dh * 4:(dh + 1) * 4, 0:64],
                        ps[:, 0:256].rearrange("p (h x) -> p h x", x=64),
                        bv_sb[:, dh * 256:(dh + 1) * 256].rearrange(
                            "p (h x) -> p h x", x=64),
                    )
                thunks.append(vst)
            return thunks

        def outproj_thunks(tq):
            thunks = []
            for tt in range(tq * 4, tq * 4 + 4):
                for c in range(2):
                    def unit(tt=tt, c=c):
                        op_ps = pmx.tile([128, 512], f32, tag="mx", name="mx_ps")
                        for p in range(NP):
                            nc.tensor.matmul(
                                op_ps,
                                Onorm[p][:, tt * 128:(tt + 1) * 128],
                                WoSs[p][:, c * 512:(c + 1) * 512],
                                start=(p == 0),
                                stop=(p == 3),
                            )
                        oc = ocp_pool.tile([128, 512], f32, tag="ocp", name="oc")
                        nc.vector.tensor_copy(oc, op_ps)
                        nc.sync.dma_start(
                            out=out_d[tt * 128:(tt + 1) * 128,
                                      c * 512:(c + 1) * 512],
                            in_=oc)
                    thunks.append(unit)
            return thunks

        def outproj_tail(tq):
            # tail variant: "sc" psum tiles are free once scoring has ended,
            # so use wide [128,1024] units to avoid mx-slot serialization
            for tt in range(tq * 4, tq * 4 + 4):
                op_ps = psc.tile([128, 1024], f32, tag="sc", name="sc_ps")
                for c in range(2):
                    for p in range(NP):
                        nc.tensor.matmul(
                            op_ps[:, c * 512:(c + 1) * 512],
                            Onorm[p][:, tt * 128:(tt + 1) * 128],
                            WoSs[p][:, c * 512:(c + 1) * 512],
                            start=(p == 0),
                            stop=(p == 3),
                        )
                oc = ocp_pool.tile([128, 1024], f32, tag="ocpw", name="ocw")
                nc.vector.tensor_copy(oc, op_ps)
                nc.sync.dma_start(out=out_d[tt * 128:(tt + 1) * 128, :], in_=oc)

        class PrevStage:
            def __init__(self, p, tq, exs):
                self.p, self.tq, self.exs = p, tq, exs
                self.pv = None      # two [128, 4*65] psum tiles (4 units each)

        def emit_pv_slot(prev, st):
            """8 accumulating matmuls: O_td[t, d]/den for each (tb, h) unit,
            with exp (s x t block) stationary and the 65-col augmented v
            moving. Unit g = tb*2 + h lives at cols (g%4)*65 of pv tile g//4;
            one start per PSUM bank (unit 0 of each tile, which pending-zeroes
            the whole bank), one stop (unit 3)."""
            if st == 0:
                prev.pv = [
                    ppv.tile([128, 260], f32, tag="pvA", name="pvA",
                             padded_shape=[128, 512]),
                    ppv.tile([128, 260], f32, tag="pvB", name="pvB",
                             padded_shape=[128, 512]),
                ]
            for g in range(8):
                tb, h = g // 2, g % 2
                pvt = prev.pv[g // 4]
                u = g % 4
                hidx = 2 * prev.p + h
                nc.tensor.matmul(
                    pvt[:, u * 65:(u + 1) * 65],
                    prev.exs[st][:, h * 512 + tb * 128:h * 512 + (tb + 1) * 128],
                    vaug[st][:, hidx * 65:hidx * 65 + 65],
                    start=(st == 0 and u == 0),
                    stop=(st == 15 and u == 3),
                )

        def emit_normalize(prev):
            """den sits on the same partition as its O row: reciprocal of the
            65th column of each unit, one tensor_tensor mul per pv tile (rc
            broadcast along d via a stride-0 AP) into a [t, d] bf16 tile, then
            one XBAR DMA transpose restoring the d-major layout out-proj
            wants (out 3D => per-128-block transpose)."""
            rc = small.tile([128, 8], f32, tag="rc", name="rc")
            otd = small.tile([128, 512], bf16, tag="otd", name="otd")
            for i in range(2):
                pv3 = prev.pv[i].rearrange("p (u x) -> p u x", x=65)
                nc.vector.reciprocal(rc[:, i * 4:(i + 1) * 4], pv3[:, :, 64:65])
            for i in range(2):
                pv3 = prev.pv[i].rearrange("p (u x) -> p u x", x=65)
                rc_b = bass.AP(
                    tensor=rc.tensor,
                    offset=rc[:, i * 4:(i + 1) * 4].offset,
                    ap=[rc.ap[0], [1, 4], [0, 64]],
                )
                nc.vector.tensor_mul(
                    otd[:, i * 256:(i + 1) * 256].rearrange(
                        "p (u x) -> p u x", x=64),
                    pv3[:, :, 0:64],
                    rc_b,
                )
            t0 = prev.tq * 512
            nc.sync.dma_start_transpose(
                out=Onorm[prev.p][:, t0:t0 + 512].rearrange(
                    "p (b t) -> p b t", t=128),
                in_=otd,
            )
            prev.pv = None

        def emit_stage(p, tq, prev, extras, dl=16, sl0=0):
            """16 score slots for (p, tq); interleave prev stage's PV and
            the extra thunks (emitted between slots sl0..sl0+dl); returns
            this stage's record."""
            t0 = tq * 512
            exs = []
            n_ex = len(extras)
            taken = 0
            for st in range(NS):
                sc_ps = psc.tile([128, 1024], f32, tag="sc", name="sc_ps")
                nc.tensor.matmul(
                    sc_ps[:, 0:512],
                    kTs[p][0:64, st * 128:(st + 1) * 128],
                    qTs[p][0:64, t0:t0 + 512],
                    start=True, stop=True,
                    tile_position=(0, 0),
                )
                nc.tensor.matmul(
                    sc_ps[:, 512:1024],
                    kTs[p][64:128, st * 128:(st + 1) * 128],
                    qTs[p][64:128, t0:t0 + 512],
                    start=True, stop=True,
                    tile_position=(64, 0),
                )
                ex = expool.tile([128, 1024], bf16, tag="ex", name="ex")
                nc.scalar.activation(ex, sc_ps, AF.Exp, scale=0.125)
                exs.append(ex)
                if st >= sl0:
                    prog = min(st - sl0 + 1, dl)
                    want = (n_ex * prog + dl - 1) // dl
                    while taken < want:
                        extras[taken]()
                        taken += 1
                if prev is not None:
                    emit_pv_slot(prev, st)
            while taken < n_ex:
                extras[taken]()
                taken += 1
            if prev is not None:
                emit_normalize(prev)
            return PrevStage(p, tq, exs)

        # ---- emission ----
        for _rep in range(repeats):
            # startup: stream in pair-0's q/k projections; the first scores
            # need q0 half0 and k0 half0 (slots 0-7). Only pair-0's weight
            # slices load up-front; the other pairs' columns follow once the
            # critical DMAs are queued.
            wq, wq_rest = load_w(WqT_d, split=True)
            xq0 = xhalf_dma(qT_d, 0)
            wk, wk_rest = load_w(WkT_d, split=True)
            xk0 = xhalf_dma(kT_d, 0)
            xk1 = xhalf_dma(kT_d, 1)           # k0 half1: needed stage-0 slot 8
            for th in proj_quarters(0, wq, lambda: xq0, qTs[0], bq_sb, 0):
                th()
            for th in proj_quarters(0, wk, lambda: xk0, kTs[0], bk_sb, 0):
                th()
            wk_rest()
            wq_rest()
            wv = load_w(WvT_d)
            xv0 = xhalf_dma(vT_d, 0)           # vpA h0: vaug st0-7 for stage-1 PV
            xq1 = xhalf_dma(qT_d, 1)           # q0 half1: needed stage 2
            for p in range(NP):
                nc.sync.dma_start(out=WoSs[p], in_=WoS_d[p * 128:(p + 1) * 128, :])

            extras = {}

            def add(sg, ths):
                extras[sg] = extras.get(sg, []) + ths

            # stage 0: k0 half1 (scores slots 8-15 of this very stage), the
            # v-projection's first s-half, and the DMA for its second
            xv1_t = dma_box(vT_d, 1)
            add(0, proj_quarters(0, wk, lambda: xk1, kTs[0], bk_sb, 1))
            add(0, [xv1_t])
            add(0, vproj_units(wv, 0, 0, lambda: xv0))
            # stage 1: q0 half1 + v-projection second s-half (vaug st8-15,
            # JIT for this stage's own PV slots)
            add(1, proj_quarters(0, wq, lambda: xq1, qTs[0], bq_sb, 1))
            add(1, vproj_units(wv, 0, 1, xv1_t.get))

            # pairs 1-3 q/k: work at stages 4p-2 (k h0), 4p-1 (q h0), 4p
            # (k h1, hard slot-8 deadline), 4p+1 (q h1); each half's two
            # DMAs issue one stage ahead.
            for p in range(1, NP):
                tk0 = dma_box(kT_d, 0)
                tq0 = dma_box(qT_d, 0)
                tk1 = dma_box(kT_d, 1)
                tq1 = dma_box(qT_d, 1)
                add(4 * p - 3, [tk0])
                add(4 * p - 2, proj_quarters(p, wk, tk0.get, kTs[p], bk_sb, 0) + [tq0])
                add(4 * p - 1, proj_quarters(p, wq, tq0.get, qTs[p], bq_sb, 0) + [tk1])
                add(4 * p, proj_quarters(p, wk, tk1.get, kTs[p], bk_sb, 1) + [tq1])
                add(4 * p + 1, proj_quarters(p, wq, tq1.get, qTs[p], bq_sb, 1))

            # second head-quad of V (pairs 2-3, needed from stage 9's PV):
            # spread over lighter stages 2/3/5/6, DMAs one stage ahead.
            xvB0_t = dma_box(vT_d, 0)
            xvB1_t = dma_box(vT_d, 1)
            add(1, [xvB0_t])
            add(2, vproj_units(wv, 1, 0, xvB0_t.get, range(0, 4)))
            add(3, vproj_units(wv, 1, 0, xvB0_t.get, range(4, 8)))
            add(4, [xvB1_t])
            add(5, vproj_units(wv, 1, 1, xvB1_t.get, range(0, 4)))
            add(6, vproj_units(wv, 1, 1, xvB1_t.get, range(4, 8)))

            add(14, outproj_thunks(0))
            add(15, outproj_thunks(1))

            # pacing deadlines: K-half1 stages (4p) must finish extras by
            # slot 8 (their own scores need those kT columns); stages 0/1
            # feed vaug just-in-time; elsewhere spread smoothly. Stages
            # 14/15's out-proj starts at slot 2 (their Onorm quarter lands
            # via the DMA transpose issued at the previous stage's end).
            dls = {0: 14, 1: 14, 4: 8, 8: 8, 12: 8, 14: 11, 15: 11}
            sl0s = {14: 4, 15: 4}
            prev = None
            for s in range(16):
                p, tq = s // 4, s % 4
                prev = emit_stage(p, tq, prev, extras.get(s, []),
                                  dl=dls.get(s, 16), sl0=sl0s.get(s, 0))

            # tail: PV of the last stage with out-proj(t2) interleaved
            # (its Onorm slices land with the transpose at the end of stage
            # 15), then the final normalize+transpose and out-proj(t3)
            op2 = outproj_thunks(2)
            taken = 0
            for st in range(NS):
                if st >= 4:
                    want = (len(op2) * (st - 3) + 11) // 12
                    while taken < want:
                        op2[taken]()
                        taken += 1
                emit_pv_slot(prev, st)
            emit_normalize(prev)
            outproj_tail(3)

    nc.compile()
    return nc


def _get_nc():
    global _cached
    if _cached is None:
        _cached = _build()
    return _cached


def _prep_core_inputs(c, query, key, value, Wq, Wk, Wv, Wo, bq, bk, bv,
                      _cache={}):
    b, g = c // 2, c % 2
    sl = slice(g * DC, (g + 1) * DC)
    key_ = (id(query), b)
    if key_ not in _cache:
        # both cores of a batch share the transposed/cast activations
        _cache.clear()
        _cache[key_] = {
            "qT": query[b].T.astype(_BF16),
            "kT": key[b].T.astype(_BF16),
            "vT": value[b].T.astype(_BF16),
        }
    shared = _cache[key_]
    return {
        **shared,
        "WqT": Wq[sl].T.astype(_BF16),
        "WkT": Wk[sl].T.astype(_BF16),
        "WvT": Wv[sl].T.astype(_BF16),
        "WoS": Wo[:, sl].T.astype(_BF16),
        "bq": np.ascontiguousarray(bq[sl].reshape(NP, 128).T),
        "bk": np.ascontiguousarray(bk[sl].reshape(NP, 128).T),
        "bv": np.ascontiguousarray(bv[sl].reshape(1, DC)),
    }


def kernel(**inputs):
    from concourse.bass_utils import run_bass_kernel_spmd

    args = {k: np.asarray(inputs[k], np.float32)
            for k in ("query", "key", "value", "Wq", "Wk", "Wv", "Wo",
                      "bq", "bk", "bv", "bo")}
    _prep_core_inputs.__defaults__[0].clear()
    nc = _get_nc()
    in_maps = [
        _prep_core_inputs(c, args["query"], args["key"], args["value"],
                          args["Wq"], args["Wk"], args["Wv"], args["Wo"],
                          args["bq"], args["bk"], args["bv"])
        for c in range(8)
    ]
    res = run_bass_kernel_spmd(nc, in_maps, core_ids=list(range(8)))
    outs = [r["out"] for r in res.results]
    final = np.empty((B, T, E), np.float32)
    for b in range(B):
        final[b] = outs[2 * b] + outs[2 * b + 1] + args["bo"][None, :]
    return final
